# revision 27
# baseline (speedup 1.0000x reference)
"""Trainium2 Bass kernel for per-expert 2-layer MLP (grouped GEMM -> GELU -> grouped GEMM).

reference: hidden = einsum('end,edh->enh', x, w1); gelu(erf); out = einsum('enh,ehd->end', h, w2)
shapes:    x [16, 2048, 1024] f32, w1 [16, 1024, 4096] f32, w2 [16, 4096, 1024] f32

Expert-parallel over 8 NeuronCores: core c owns experts [2c, 2c+1], no
cross-core communication.  Per core, per expert:

  phase A:  actT[h, n] = gelu(w1[d, h].T @ xT[d, n])   (PE matmul, contraction d)
  phase B:  out[n, d'] = actT[h, n].T @ w2[h, d']      (PE matmul, contraction h)

Matmul1 with w1 stationary directly yields hidden TRANSPOSED ([h, n]), which is
exactly the lhsT layout matmul2 needs.  All operands are pre-cast to fp16 and
pre-permuted on the host so that every device DMA moves 128 fat contiguous
per-partition segments (2-16KB descriptors):

  w1 host layout [P, HB, KD, 128]: line p = w1[k*128+p, hb*128+c], hb-major.
    An hb-range DMA is 128 x (range*2KB) contiguous.
  w2 host layout [P, KH, D]:       line p = w2[h*128+p, d], h-major.
  x  host layout [NBLK, P, KD, NB]: line p = x[nb*512+n, k*128+p] transposed.

Engine queues: GpSimd triggers all weight DMAs (FIFO order doubles as the
bandwidth priority: w1-e0 chunks first, then gated w2), Sync/Scalar trigger
x loads and output stores, Vector does PSUM->SBUF fp16 copies, Scalar runs
GELU.  A burst of dummy matmuls at the start keeps the PE busy while the
first DMAs land: the DVFS ramp (~3us of continuous execution to reach
2.4GHz, and any >~1us idle gap resets it) completes during the DMA wait,
so every real matmul runs at the 216ns/512-row instruction floor.
"""

import os
import sys

import numpy as np

for _p in ("/opt/trn_rl_repo", "/root/.axon_site/_ro/trn_rl_repo"):
    if os.path.isdir(_p) and _p not in sys.path:
        sys.path.append(_p)

import concourse.bacc as bacc
import concourse.tile as tile
from concourse import mybir
from concourse.bass_utils import run_bass_kernel_spmd

E, N, D, H = 16, 2048, 1024, 4096
NCORES = 8
EPC = E // NCORES        # experts per core                     = 2
P = 128                  # SBUF partitions
FD = 512                 # matmul moving free dim
NB = 512                 # token block per phase-A/B iteration
N_BLOCKS = N // NB       # = 4
N_SUB = NB // P          # row sub-blocks per token block       = 4
KD = D // P              # d-blocks (contraction of matmul 1)   = 8
KH = H // P              # h-blocks (contraction of matmul 2)   = 32
HB = H // P              # h-block count for w1 layout          = 32
DC = D // FD             # d' chunks (free dim of matmul 2)     = 2
NWARM = 17               # PE clock warm-up dummy matmuls (512-row)
NWARM_SMALL = 6          # fine-grained 64-row cushion dummies
F16 = mybir.dt.float16
F32 = mybir.dt.float32

_CACHE = {}


def _build():
    nc = bacc.Bacc(None, target_bir_lowering=False)
    xt_d = nc.declare_dram_parameter("xt", [EPC, N_BLOCKS, P, KD * NB], F16, isOutput=False)
    w1_d = nc.declare_dram_parameter("w1", [EPC, P, HB * KD * P], F16, isOutput=False)
    w2_d = nc.declare_dram_parameter("w2", [EPC, P, KH * D], F16, isOutput=False)
    out_d = nc.declare_dram_parameter("out", [EPC, N, D], F16, isOutput=True)

    with (
        tile.TileContext(nc) as tc,
        tc.tile_pool(name="warm", bufs=1) as warm_pool,
        tc.tile_pool(name="w1sb", bufs=1) as w1_pool,
        tc.tile_pool(name="w2sb", bufs=1) as w2_pool,
        tc.tile_pool(name="xT", bufs=2) as xt_pool,
        tc.tile_pool(name="actT", bufs=1) as act_pool,
        tc.tile_pool(name="osb", bufs=3) as out_pool,
        tc.tile_pool(name="ps_1", bufs=4, space="PSUM") as ps1_pool,
        tc.tile_pool(name="ps_2", bufs=4, space="PSUM") as ps2_pool,
    ):

        def emit_w1_loads(e, bounds, w1_sb=None):
            """hb-range chunks, 128 contiguous segments each (>=4KB)."""
            if w1_sb is None:
                w1_sb = w1_pool.tile([P, HB, KD, P], F16, tag="w1")
            w1_view = w1_d[e].rearrange("p (hb k c) -> p hb k c", hb=HB, k=KD)
            for lo, hi in zip(bounds, bounds[1:]):
                nc.gpsimd.dma_start(out=w1_sb[:, lo:hi], in_=w1_view[:, lo:hi])
            return w1_sb

        def emit_w2_loads(e):
            w2_sb = w2_pool.tile([P, KH, D], F16, tag="w2")
            w2_view = w2_d[e].rearrange("p (h d) -> p h d", h=KH)
            HBC = KH // 4
            for c in range(4):
                nc.gpsimd.dma_start(
                    out=w2_sb[:, c * HBC : (c + 1) * HBC, :],
                    in_=w2_view[:, c * HBC : (c + 1) * HBC, :],
                )
            return w2_sb

        def emit_x_loads(e, nb, eng=None):
            # nb>=1 loads ride the Scalar ring: the gelu stream ahead of them
            # paces their descriptor-gen to one block early, keeping the
            # latency-critical startup rings (sync/gpsimd) uncontended.
            xt_sb = xt_pool.tile([P, KD, NB], F16, tag="xT")
            xt_view = xt_d[e, nb].rearrange("p (k n) -> p k n", k=KD)
            (eng or nc.scalar).dma_start(out=xt_sb[:, :, :], in_=xt_view[:, :, :])
            return xt_sb

        def emit_warmup():
            """Dummy matmuls on a zeroed tile: ramp the PE clock (DVFS takes
            ~3us of continuous execution) while the first w1/x DMAs land."""
            warm = warm_pool.tile([P, NB], F16, tag="warm")
            nc.gpsimd.memset(warm, 0.0)
            for _ in range(NWARM):
                pw = ps1_pool.tile([P, NB], F32, tag="ps1")
                nc.tensor.matmul(pw, lhsT=warm[:, 0:P], rhs=warm, start=True, stop=True)
            for _ in range(NWARM_SMALL):
                pw = ps1_pool.tile([P, NB], F32, tag="ps1")
                nc.tensor.matmul(pw[:, 0:64], lhsT=warm[:, 0:P], rhs=warm[:, 0:64],
                                 start=True, stop=True)

        def emit_phase_a(w1_sb, xt_sb):
            actT = act_pool.tile([P, KH, NB], F16, tag="actT")
            for h in range(KH):
                ps1 = ps1_pool.tile([P, NB], F32, tag="ps1")
                for k in range(KD):
                    nc.tensor.matmul(
                        ps1, lhsT=w1_sb[:, h, k, :], rhs=xt_sb[:, k, :],
                        start=(k == 0), stop=(k == KD - 1),
                    )
                nc.scalar.activation(actT[:, h, :], ps1, mybir.ActivationFunctionType.Gelu)
            return actT

        def emit_phase_b(e, nb, actT, w2_sb, last):
            n0 = nb * NB
            for s in range(N_SUB):
                osb = out_pool.tile([P, D], F16, tag="osb")
                split = last and s == N_SUB - 1
                for c in range(DC):
                    ps2 = ps2_pool.tile([P, FD], F32, tag="ps2")
                    for h in range(KH):
                        nc.tensor.matmul(
                            ps2, lhsT=actT[:, h, s * P : (s + 1) * P],
                            rhs=w2_sb[:, h, c * FD : (c + 1) * FD],
                            start=(h == 0), stop=(h == KH - 1),
                        )
                    if split:
                        # tail: store each half on its own idle ring so the
                        # c=0 store overlaps the c=1 chain+cast
                        seng = nc.sync if c == 0 else nc.scalar
                        nc.vector.tensor_copy(osb[:, c * FD : (c + 1) * FD], ps2)
                        seng.dma_start(
                            out=out_d[e, n0 + s * P : n0 + (s + 1) * P,
                                      c * FD : (c + 1) * FD],
                            in_=osb[:, c * FD : (c + 1) * FD],
                        )
                    else:
                        nc.vector.tensor_copy(osb[:, c * FD : (c + 1) * FD], ps2)
                if not split:
                    nc.sync.dma_start(
                        out=out_d[e, n0 + s * P : n0 + (s + 1) * P, :], in_=osb
                    )

        emit_warmup()
        # Expert-0 startup.  The early DMA phase is latency-bound (~300ns per
        # descriptor per channel, size-independent up to the ~358GB/s BW cap,
        # round-robin across rings, FIFO within ring).  First wave: the whole
        # first x block (sync ring, 128x8KB) and w1 h-blocks 0-1 (gpsimd);
        # later w1 chunks stream in consumption order with nothing else
        # queued on either ring.
        w1_cur = emit_w1_loads(0, [0, 4, 8, 16, 32])
        xt_sb0 = emit_x_loads(0, 0, eng=nc.sync)
        w1_next = None
        w2_cur = None
        for e in range(EPC):
            for nb in range(N_BLOCKS):
                xt_sb = xt_sb0 if (e == 0 and nb == 0) else emit_x_loads(e, nb)
                actT = emit_phase_a(w1_cur, xt_sb)
                if nb == 0:
                    if e == 0:
                        # Stall the w2 stream (same GpSimd ring, FIFO) until
                        # phase A is underway so the critical w1 stream keeps
                        # the HBM window to itself.
                        gate = w2_pool.tile([P, 4], F32, tag="w2")
                        nc.gpsimd.tensor_copy(gate, actT[:, 4, 0:4])
                    w2_cur = emit_w2_loads(e)
                if nb == N_BLOCKS - 1 and e + 1 < EPC:
                    w1_next = emit_w1_loads(e + 1, [0, 8, 16, 24, 32])
                emit_phase_b(e, nb, actT, w2_cur,
                             last=(e == EPC - 1 and nb == N_BLOCKS - 1))
            w1_cur = w1_next

    nc.compile()
    return nc


def _get_nc():
    if "nc" not in _CACHE:
        _CACHE["nc"] = _build()
    return _CACHE["nc"]


def _prep(inputs):
    x = np.asarray(inputs["x"], dtype=np.float32).astype(np.float16)
    w1 = np.asarray(inputs["w1"], dtype=np.float32).astype(np.float16)
    w2 = np.asarray(inputs["w2"], dtype=np.float32).astype(np.float16)
    # x [E,N,D] -> [E, NBLK, P, KD*NB]; line p = x[nb*512+n', k*128+p]
    xt = np.ascontiguousarray(
        x.reshape(E, N_BLOCKS, NB, KD, P).transpose(0, 1, 4, 3, 2)
    ).reshape(E, N_BLOCKS, P, KD * NB)
    # w1 [E,D,H] -> [E, P, HB*KD*128]; line p = w1[k*128+p, hb*128+c], hb-major
    w1p = np.ascontiguousarray(
        w1.reshape(E, KD, P, HB, P).transpose(0, 2, 3, 1, 4)
    ).reshape(E, P, HB * KD * P)
    # w2 [E,H,D] -> [E, P, KH*D]; line p = w2[h*128+p, d], h-major
    w2p = np.ascontiguousarray(
        w2.reshape(E, KH, P, D).transpose(0, 2, 1, 3)
    ).reshape(E, P, KH * D)
    return xt, w1p, w2p


def _run(inputs, trace=False, trace_cores=None):
    xt, w1p, w2p = _prep(inputs)
    nc = _get_nc()
    in_maps = [
        {
            "xt": xt[c * EPC : (c + 1) * EPC],
            "w1": w1p[c * EPC : (c + 1) * EPC],
            "w2": w2p[c * EPC : (c + 1) * EPC],
        }
        for c in range(NCORES)
    ]
    res = run_bass_kernel_spmd(
        nc, in_maps, list(range(NCORES)), trace=trace, trace_cores=trace_cores
    )
    out = np.concatenate([res.results[c]["out"] for c in range(NCORES)], axis=0)
    return out.astype(np.float32), res


def kernel(**inputs) -> np.ndarray:
    out, _ = _run(inputs, trace=False)
    return out


# revision 30
# speedup vs baseline: 1.0045x; 1.0045x over previous
"""Trainium2 Bass kernel for per-expert 2-layer MLP (grouped GEMM -> GELU -> grouped GEMM).

reference: hidden = einsum('end,edh->enh', x, w1); gelu(erf); out = einsum('enh,ehd->end', h, w2)
shapes:    x [16, 2048, 1024] f32, w1 [16, 1024, 4096] f32, w2 [16, 4096, 1024] f32

Expert-parallel over 8 NeuronCores: core c owns experts [2c, 2c+1], no
cross-core communication.  Per core, per expert:

  phase A:  actT[h, n] = gelu(w1[d, h].T @ xT[d, n])   (PE matmul, contraction d)
  phase B:  out[n, d'] = actT[h, n].T @ w2[h, d']      (PE matmul, contraction h)

Matmul1 with w1 stationary directly yields hidden TRANSPOSED ([h, n]), which is
exactly the lhsT layout matmul2 needs.  All operands are pre-cast to fp16 and
pre-permuted on the host so that every device DMA moves 128 fat contiguous
per-partition segments (2-16KB descriptors):

  w1 host layout [P, HB, KD, 128]: line p = w1[k*128+p, hb*128+c], hb-major.
    An hb-range DMA is 128 x (range*2KB) contiguous.
  w2 host layout [P, KH, D]:       line p = w2[h*128+p, d], h-major.
  x  host layout [NBLK, P, KD, NB]: line p = x[nb*512+n, k*128+p] transposed.

Engine queues: GpSimd triggers all weight DMAs (FIFO order doubles as the
bandwidth priority: w1-e0 chunks first, then gated w2), Sync/Scalar trigger
x loads and output stores, Vector does PSUM->SBUF fp16 copies, Scalar runs
GELU.  A burst of dummy matmuls at the start keeps the PE busy while the
first DMAs land: the DVFS ramp (~3us of continuous execution to reach
2.4GHz, and any >~1us idle gap resets it) completes during the DMA wait,
so every real matmul runs at the 216ns/512-row instruction floor.
"""

import os
import sys

import numpy as np

for _p in ("/opt/trn_rl_repo", "/root/.axon_site/_ro/trn_rl_repo"):
    if os.path.isdir(_p) and _p not in sys.path:
        sys.path.append(_p)

import concourse.bacc as bacc
import concourse.tile as tile
from concourse import mybir
from concourse.bass_utils import run_bass_kernel_spmd

E, N, D, H = 16, 2048, 1024, 4096
NCORES = 8
EPC = E // NCORES        # experts per core                     = 2
P = 128                  # SBUF partitions
FD = 512                 # matmul moving free dim
NB = 512                 # token block per phase-A/B iteration
N_BLOCKS = N // NB       # = 4
N_SUB = NB // P          # row sub-blocks per token block       = 4
KD = D // P              # d-blocks (contraction of matmul 1)   = 8
KH = H // P              # h-blocks (contraction of matmul 2)   = 32
HB = H // P              # h-block count for w1 layout          = 32
DC = D // FD             # d' chunks (free dim of matmul 2)     = 2
NWARM = 33               # PE clock warm-up dummy matmuls (512-row)
NWARM_SMALL = 6          # fine-grained 64-row cushion dummies
F16 = mybir.dt.float16
F32 = mybir.dt.float32

_CACHE = {}


def _build():
    nc = bacc.Bacc(None, target_bir_lowering=False)
    xt_d = nc.declare_dram_parameter("xt", [EPC, N_BLOCKS, P, KD * NB], F16, isOutput=False)
    w1_d = nc.declare_dram_parameter("w1", [EPC, P, HB * KD * P], F16, isOutput=False)
    w2_d = nc.declare_dram_parameter("w2", [EPC, P, KH * D], F16, isOutput=False)
    out_d = nc.declare_dram_parameter("out", [EPC, N, D], F16, isOutput=True)

    with (
        tile.TileContext(nc) as tc,
        tc.tile_pool(name="warm", bufs=1) as warm_pool,
        tc.tile_pool(name="w1sb", bufs=1) as w1_pool,
        tc.tile_pool(name="w2sb", bufs=1) as w2_pool,
        tc.tile_pool(name="xT", bufs=2) as xt_pool,
        tc.tile_pool(name="actT", bufs=1) as act_pool,
        tc.tile_pool(name="osb", bufs=3) as out_pool,
        tc.tile_pool(name="ps_1", bufs=4, space="PSUM") as ps1_pool,
        tc.tile_pool(name="ps_2", bufs=4, space="PSUM") as ps2_pool,
    ):

        def emit_w1_loads(e, bounds, w1_sb=None):
            """hb-range chunks, 128 contiguous segments each (>=4KB)."""
            if w1_sb is None:
                w1_sb = w1_pool.tile([P, HB, KD, P], F16, tag="w1")
            w1_view = w1_d[e].rearrange("p (hb k c) -> p hb k c", hb=HB, k=KD)
            for lo, hi in zip(bounds, bounds[1:]):
                nc.gpsimd.dma_start(out=w1_sb[:, lo:hi], in_=w1_view[:, lo:hi])
            return w1_sb

        def emit_w2_loads(e):
            w2_sb = w2_pool.tile([P, KH, D], F16, tag="w2")
            w2_view = w2_d[e].rearrange("p (h d) -> p h d", h=KH)
            HBC = KH // 4
            for c in range(4):
                nc.gpsimd.dma_start(
                    out=w2_sb[:, c * HBC : (c + 1) * HBC, :],
                    in_=w2_view[:, c * HBC : (c + 1) * HBC, :],
                )
            return w2_sb

        def emit_x_loads(e, nb, eng=None):
            # nb>=1 loads ride the Scalar ring: the gelu stream ahead of them
            # paces their descriptor-gen to one block early, keeping the
            # latency-critical startup rings (sync/gpsimd) uncontended.
            xt_sb = xt_pool.tile([P, KD, NB], F16, tag="xT")
            xt_view = xt_d[e, nb].rearrange("p (k n) -> p k n", k=KD)
            (eng or nc.scalar).dma_start(out=xt_sb[:, :, :], in_=xt_view[:, :, :])
            return xt_sb

        def emit_warmup():
            """Dummy matmuls on a zeroed tile: ramp the PE clock (DVFS takes
            ~3us of continuous execution) while the first w1/x DMAs land."""
            warm = warm_pool.tile([P, NB], F16, tag="warm")
            nc.gpsimd.memset(warm, 0.0)
            for _ in range(NWARM):
                pw = ps1_pool.tile([P, NB], F32, tag="ps1")
                nc.tensor.matmul(pw, lhsT=warm[:, 0:P], rhs=warm, start=True, stop=True)
            for _ in range(NWARM_SMALL):
                pw = ps1_pool.tile([P, NB], F32, tag="ps1")
                nc.tensor.matmul(pw[:, 0:64], lhsT=warm[:, 0:P], rhs=warm[:, 0:64],
                                 start=True, stop=True)

        def emit_phase_a(w1_sb, xt_sb):
            actT = act_pool.tile([P, KH, NB], F16, tag="actT")
            for h in range(KH):
                ps1 = ps1_pool.tile([P, NB], F32, tag="ps1")
                for k in range(KD):
                    nc.tensor.matmul(
                        ps1, lhsT=w1_sb[:, h, k, :], rhs=xt_sb[:, k, :],
                        start=(k == 0), stop=(k == KD - 1),
                    )
                nc.scalar.activation(actT[:, h, :], ps1, mybir.ActivationFunctionType.Gelu)
            return actT

        def emit_phase_b(e, nb, actT, w2_sb, last):
            n0 = nb * NB
            for s in range(N_SUB):
                osb = out_pool.tile([P, D], F16, tag="osb")
                split = last and s == N_SUB - 1
                # final sub-block: progressively smaller chunks (512/256/256)
                # so each cast+store overlaps the next chain and the very
                # last cast+store is half-sized
                chunks = [(0, FD), (FD, FD + 256), (FD + 256, D)] if split \
                    else [(0, FD), (FD, D)]
                for ci, (d0, d1) in enumerate(chunks):
                    ps2 = ps2_pool.tile([P, FD], F32, tag="ps2")
                    for h in range(KH):
                        nc.tensor.matmul(
                            ps2[:, 0 : d1 - d0],
                            lhsT=actT[:, h, s * P : (s + 1) * P],
                            rhs=w2_sb[:, h, d0:d1],
                            start=(h == 0), stop=(h == KH - 1),
                        )
                    nc.vector.tensor_copy(osb[:, d0:d1], ps2[:, 0 : d1 - d0])
                    if split:
                        # store each chunk on its own ring immediately
                        seng = (nc.sync, nc.sync, nc.scalar)[ci]
                        seng.dma_start(
                            out=out_d[e, n0 + s * P : n0 + (s + 1) * P, d0:d1],
                            in_=osb[:, d0:d1],
                        )
                if not split:
                    nc.sync.dma_start(
                        out=out_d[e, n0 + s * P : n0 + (s + 1) * P, :], in_=osb
                    )

        emit_warmup()
        # Expert-0 startup.  The early DMA phase is latency-bound (~300ns per
        # descriptor per channel, size-independent up to the ~358GB/s BW cap,
        # round-robin across rings, FIFO within ring).  First wave: the whole
        # first x block (sync ring, 128x8KB) and w1 h-blocks 0-1 (gpsimd);
        # later w1 chunks stream in consumption order with nothing else
        # queued on either ring.
        w1_cur = emit_w1_loads(0, [0, 2, 4, 8, 16, 32])
        xt_sb0 = emit_x_loads(0, 0, eng=nc.sync)
        w1_next = None
        w2_cur = None
        for e in range(EPC):
            for nb in range(N_BLOCKS):
                xt_sb = xt_sb0 if (e == 0 and nb == 0) else emit_x_loads(e, nb)
                actT = emit_phase_a(w1_cur, xt_sb)
                if nb == 0:
                    if e == 0:
                        # Stall the w2 stream (same GpSimd ring, FIFO) until
                        # phase A is underway so the critical w1 stream keeps
                        # the HBM window to itself.
                        gate = w2_pool.tile([P, 4], F32, tag="w2")
                        nc.gpsimd.tensor_copy(gate, actT[:, 4, 0:4])
                    w2_cur = emit_w2_loads(e)
                if nb == N_BLOCKS - 1 and e + 1 < EPC:
                    w1_next = emit_w1_loads(e + 1, [0, 8, 16, 24, 32])
                emit_phase_b(e, nb, actT, w2_cur,
                             last=(e == EPC - 1 and nb == N_BLOCKS - 1))
            w1_cur = w1_next

    nc.compile()
    return nc


def _get_nc():
    if "nc" not in _CACHE:
        _CACHE["nc"] = _build()
    return _CACHE["nc"]


def _prep(inputs):
    x = np.asarray(inputs["x"], dtype=np.float32).astype(np.float16)
    w1 = np.asarray(inputs["w1"], dtype=np.float32).astype(np.float16)
    w2 = np.asarray(inputs["w2"], dtype=np.float32).astype(np.float16)
    # x [E,N,D] -> [E, NBLK, P, KD*NB]; line p = x[nb*512+n', k*128+p]
    xt = np.ascontiguousarray(
        x.reshape(E, N_BLOCKS, NB, KD, P).transpose(0, 1, 4, 3, 2)
    ).reshape(E, N_BLOCKS, P, KD * NB)
    # w1 [E,D,H] -> [E, P, HB*KD*128]; line p = w1[k*128+p, hb*128+c], hb-major
    w1p = np.ascontiguousarray(
        w1.reshape(E, KD, P, HB, P).transpose(0, 2, 3, 1, 4)
    ).reshape(E, P, HB * KD * P)
    # w2 [E,H,D] -> [E, P, KH*D]; line p = w2[h*128+p, d], h-major
    w2p = np.ascontiguousarray(
        w2.reshape(E, KH, P, D).transpose(0, 2, 1, 3)
    ).reshape(E, P, KH * D)
    return xt, w1p, w2p


def _run(inputs, trace=False, trace_cores=None):
    xt, w1p, w2p = _prep(inputs)
    nc = _get_nc()
    in_maps = [
        {
            "xt": xt[c * EPC : (c + 1) * EPC],
            "w1": w1p[c * EPC : (c + 1) * EPC],
            "w2": w2p[c * EPC : (c + 1) * EPC],
        }
        for c in range(NCORES)
    ]
    res = run_bass_kernel_spmd(
        nc, in_maps, list(range(NCORES)), trace=trace, trace_cores=trace_cores
    )
    out = np.concatenate([res.results[c]["out"] for c in range(NCORES)], axis=0)
    return out.astype(np.float32), res


def kernel(**inputs) -> np.ndarray:
    out, _ = _run(inputs, trace=False)
    return out
